# revision 15
# baseline (speedup 1.0000x reference)
"""Distributed attention kernel for 8 trn2 NeuronCores.

Reference semantics (B=2, S=2048, D=2048, H=16, dh=128):
  q = x@W_q, k = x@W_k  (per-head split), v = x@W_v (full width)
  scores = q@k^T per head; (scores + triu(-1e9)) * 1/sqrt(dh); softmax
  out = (sum_h probs_h) @ v @ W_o        <- heads summed, v full width

Sharding: 2 groups of 4 cores (batch parallel); within a group, rank r
projects heads {4r..4r+3} of k (cols of W_k) and cols [512r, 512r+512)
of W_v.  kT is AllGathered (f32) and v is AllGathered (bf16).  Each
rank computes qT for all 16 heads of its own 4 interleaved q-tiles
{r, 4+r, 8+r, 12+r} (load-balanced under the causal mask) locally
from host-gathered inputs x_own (own 512 q-columns of x^T) and wqf
(full W_q), then computes scores/softmax/P/(P@v)/W_o end-to-end for
its own 512 q rows.  P never leaves SBUF: summed over all 16 heads
in PSUM via diag(1/rowsum) matmuls, PE-transposed, then contracted
with v and W_o.  No ReduceScatter, no P round-trip.

The program is rank-agnostic (required: one NEFF for all 8 cores).
Rank enters only through input data: which W_k/W_v columns are fed,
x_own, and the causal `mask` tensor (diag block at col 128*r of the
last 512-chunk).  kwc(own[s]) = s+1 for every rank, so causal loop
bounds are identical across ranks.

Precision: score path (x@Wq, x@Wk, q@k^T) in float32r (TF32-like,
full PE rate at N>=256); softmax in f32; E/P in bf16; v/P^T/O/W_o in
bf16 with f32 PSUM accumulation.
"""

import math

import numpy as np
import ml_dtypes

import concourse.bass as bass
import concourse.mybir as mybir
import concourse.tile as tile
from concourse import bacc
from concourse.bass_utils import run_bass_kernel_spmd
from concourse.masks import make_identity

F32 = mybir.dt.float32
F32R = mybir.dt.float32r
BF16 = mybir.dt.bfloat16

S = 2048
D = 2048
DH = 128
NT = S // 128  # 16 q/k tiles
SCALE = 1.0 / math.sqrt(DH)
GROUPS = [[0, 1, 2, 3], [4, 5, 6, 7]]
NEG = -1e9


def build():
    nc = bacc.Bacc("TRN2", target_bir_lowering=False, debug=False, num_devices=8)

    x = nc.declare_dram_parameter("x", [D, S], F32R, isOutput=False)  # x TRANSPOSED
    xbf = nc.declare_dram_parameter("xbf", [D, S], BF16, isOutput=False)
    x_own = nc.declare_dram_parameter("x_own", [D, 512], F32R, isOutput=False)
    wqf = nc.declare_dram_parameter("wqf", [D, D], F32R, isOutput=False)
    wk = nc.declare_dram_parameter("wk", [D, 512], F32R, isOutput=False)
    wv = nc.declare_dram_parameter("wv", [D, 512], BF16, isOutput=False)
    wo = nc.declare_dram_parameter("wo", [D, D], BF16, isOutput=False)
    mask_in = nc.declare_dram_parameter("mask", [128, 512], F32R, isOutput=False)
    out = nc.declare_dram_parameter("out", [512, D], F32, isOutput=True)

    # internal DRAM for collectives
    kt_l = nc.dram_tensor("kt_l", [128, 4 * S], F32R)  # [dh, hh*S + k]
    kt_ag = nc.dram_tensor("kt_ag", [4, 128, 4 * S], F32R)
    v_local = nc.dram_tensor("v_local", [S, 512], BF16)
    v_ag = nc.dram_tensor("v_ag", [4, S, 512], BF16)

    with tile.TileContext(nc) as tc:
        with tc.tile_pool(name="const", bufs=1) as cst:
            ident = cst.tile([128, 128], F32)
            make_identity(nc, ident)
            ident_bf = cst.tile([128, 128], BF16)
            nc.vector.tensor_copy(out=ident_bf[:], in_=ident[:])
            mask32 = cst.tile([128, 512], F32R)
            nc.sync.dma_start(mask32[:], mask_in[:])
            mask_bf = cst.tile([128, 512], BF16)
            nc.vector.tensor_copy(out=mask_bf[:], in_=mask32[:])
            # P^T for own q columns, bf16: [k-part, kt, own-q(slab-ordered)]
            pt = cst.tile([128, NT, 512], BF16)
            # P (pre-transpose), per own slab s, exact causal width 512*(s+1)
            P_sb = [cst.tile([128, 512 * (s + 1)], BF16, name=f"Psb{s}") for s in range(4)]

            # qT for own q rows, all 16 heads: [dh-part, h, own-q(slab order)]
            qT_own = cst.tile([128, 16, 512], F32R, name="qT_own")

            # ---------------- Phase A: k projection ----------------
            with (
                tc.tile_pool(name="xt_pool", bufs=1) as xtp,
                tc.tile_pool(name="w_pool", bufs=1) as wpp,
                tc.tile_pool(name="stage", bufs=6) as stp,
                tc.tile_pool(name="ab_ps", bufs=8, space="PSUM") as pjp,
            ):
                wk_sb = wpp.tile([128, NT, 512], F32R, name="wk_sb")
                for Dt in range(NT):
                    nc.sync.dma_start(wk_sb[:, Dt, :], wk[Dt * 128 : (Dt + 1) * 128, :])
                for sh in range(2):  # S half
                    s0 = sh * 1024
                    xt = xtp.tile([128, NT, 1024], F32R, tag="xt", name=f"xt{sh}")
                    psums = [
                        pjp.tile([128, 512], F32, tag="ps512", name=f"proj{_j}")
                        for _j in range(8)
                    ]
                    for Dt in range(NT):
                        nc.sync.dma_start(
                            xt[:, Dt, :],
                            x[Dt * 128 : (Dt + 1) * 128, s0 : s0 + 1024],
                        )
                        for j in range(8):
                            hh, qc = divmod(j, 2)
                            nc.tensor.matmul(
                                psums[j][:],
                                wk_sb[:, Dt, hh * 128 : (hh + 1) * 128],
                                xt[:, Dt, qc * 512 : (qc + 1) * 512],
                                start=(Dt == 0),
                                stop=(Dt == NT - 1),
                            )
                    for j in range(8):
                        hh, qc = divmod(j, 2)
                        st = stp.tile([128, 512], F32R, tag="st")
                        nc.vector.tensor_copy(out=st[:], in_=psums[j][:])
                        nc.sync.dma_start(
                            kt_l[
                                :,
                                hh * S + s0 + qc * 512 : hh * S + s0 + (qc + 1) * 512,
                            ],
                            st[:],
                        )

            nc.gpsimd.collective_compute(
                "AllGather",
                mybir.AluOpType.bypass,
                ins=[kt_l[:]],
                outs=[kt_ag[:]],
                replica_groups=GROUPS,
            )

            # ---------------- Phase Q: local q projection ----------------
            # qT_own[dh, h, s*128+qq] = sum_D wqf[D, h*128+dh] * x_own[D, s*128+qq]
            with (
                tc.tile_pool(name="xo_pool", bufs=1) as xop,
                tc.tile_pool(name="wqf_pool", bufs=6) as wfp,
                tc.tile_pool(name="q_ps", bufs=8, space="PSUM") as qps,
            ):
                xo = xop.tile([128, NT, 512], F32R, name="xo")
                for Dt in range(NT):
                    nc.sync.dma_start(xo[:, Dt, :], x_own[Dt * 128 : (Dt + 1) * 128, :])
                for hp in range(4):  # 4 heads per pass; 4 banks live + 4 draining
                    psums = [
                        qps.tile([128, 512], F32, tag="qp", name=f"qproj{hp}_{_j}")
                        for _j in range(4)
                    ]
                    for Dt in range(NT):
                        wf_t = wfp.tile([128, 512], F32R, tag="wf")
                        nc.sync.dma_start(
                            wf_t[:],
                            wqf[
                                Dt * 128 : (Dt + 1) * 128,
                                hp * 512 : (hp + 1) * 512,
                            ],
                        )
                        for j in range(4):
                            nc.tensor.matmul(
                                psums[j][:],
                                wf_t[:, j * 128 : (j + 1) * 128],
                                xo[:, Dt, :],
                                start=(Dt == 0),
                                stop=(Dt == NT - 1),
                            )
                    for j in range(4):
                        h = hp * 4 + j
                        nc.vector.tensor_copy(out=qT_own[:, h, :], in_=psums[j][:])

            # ---------------- Phase B: v projection ----------------
            with (
                tc.tile_pool(name="wv_pool", bufs=1) as wvp,
                tc.tile_pool(name="bstage", bufs=6) as bstp,
                tc.tile_pool(name="b_ps", bufs=8, space="PSUM") as bjp,
            ):
                wv_sb = wvp.tile([128, NT, 512], BF16, name="wv_sb")
                for Dt in range(NT):
                    nc.sync.dma_start(wv_sb[:, Dt, :], wv[Dt * 128 : (Dt + 1) * 128, :])
                for sh in range(2):
                    s0 = sh * 1024
                    psums = [
                        bjp.tile([128, 512], F32, tag="ps512", name=f"projv{_j}")
                        for _j in range(8)
                    ]
                    for Dt in range(NT):
                        xb_t = bstp.tile([128, 1024], BF16, tag="xb")
                        nc.sync.dma_start(
                            xb_t[:], xbf[Dt * 128 : (Dt + 1) * 128, s0 : s0 + 1024]
                        )
                        for j in range(8):
                            nc.tensor.matmul(
                                psums[j][:],
                                xb_t[:, j * 128 : (j + 1) * 128],
                                wv_sb[:, Dt, :],
                                start=(Dt == 0),
                                stop=(Dt == NT - 1),
                            )
                    for j in range(8):
                        kt = sh * 8 + j
                        v_sb = bstp.tile([128, 512], BF16, tag="vsb")
                        nc.vector.tensor_copy(out=v_sb[:], in_=psums[j][:])
                        nc.sync.dma_start(v_local[kt * 128 : (kt + 1) * 128, :], v_sb[:])

            nc.gpsimd.collective_compute(
                "AllGather",
                mybir.AluOpType.bypass,
                ins=[v_local[:]],
                outs=[v_ag[:]],
                replica_groups=GROUPS,
            )

            # ---------------- Phase C: scores / softmax / P ----------------
            with (
                tc.tile_pool(name="kt_pool", bufs=1) as ktp,
                tc.tile_pool(name="epool", bufs=2) as ep,
                tc.tile_pool(name="small", bufs=32) as smp,
                tc.tile_pool(name="dsm", bufs=4) as dsm,
                tc.tile_pool(name="sc_ps", bufs=2, space="PSUM") as scp,
            ):
                kT = ktp.tile([128, 16, S], F32R, name="kT_sb")
                for kc in range(4):
                    for hg in range(4):
                        for hh in range(4):
                            nc.scalar.dma_start(
                                kT[:, 4 * hg + hh, kc * 512 : (kc + 1) * 512],
                                kt_ag[hg][:, hh * S + kc * 512 : hh * S + (kc + 1) * 512],
                            )
                for s in range(4):
                    kwc = s + 1
                    kw = 512 * kwc
                    ntile = (kw + 1023) // 1024
                    ppp_cm = tc.tile_pool(name=f"p_ps{s}", bufs=1, space="PSUM")
                    ppp = ppp_cm.__enter__()
                    P_acc = ppp.tile([128, kw], F32, tag="P", name=f"Pacc{s}")
                    pend_diag = None
                    for h in range(16):
                        e_t = ep.tile([128, 2048], BF16, tag="E")
                        s_tiles = [
                            scp.tile([128, 1024], F32, tag="S", name=f"sch{_j}")
                            for _j in range(ntile)
                        ]
                        dtid = (kw - 512) // 1024
                        doff = (kw - 512) % 1024
                        for kc in range(kwc):
                            last = kc == kwc - 1
                            nc.tensor.matmul(
                                s_tiles[kc // 2][
                                    :, (kc % 2) * 512 : (kc % 2) * 512 + 512
                                ],
                                qT_own[:, h, s * 128 : (s + 1) * 128],
                                kT[:, h, kc * 512 : (kc + 1) * 512],
                                start=True,
                                stop=not last,
                            )
                            if last:
                                # causal mask folded into the PSUM chain
                                nc.tensor.matmul(
                                    s_tiles[dtid][:, doff : doff + 512],
                                    ident_bf[:],
                                    mask_bf[:],
                                    start=False,
                                    stop=True,
                                    skip_group_check=True,
                                )
                        mxp = smp.tile([128, 4], F32, tag="mx4")
                        for ti in range(ntile):
                            w = min(kw - 1024 * ti, 1024)
                            nc.vector.reduce_max(
                                out=mxp[:, ti : ti + 1],
                                in_=s_tiles[ti][:, :w],
                                axis=mybir.AxisListType.X,
                            )
                        nmS = smp.tile([128, 1], F32, tag="mx")
                        if ntile > 1:
                            mx = smp.tile([128, 1], F32, tag="mx")
                            nc.vector.reduce_max(
                                out=mx[:], in_=mxp[:, :ntile], axis=mybir.AxisListType.X
                            )
                        else:
                            mx = mxp[:, 0:1]
                        nc.vector.tensor_scalar_mul(nmS[:], mx[:], -SCALE)
                        rcp = smp.tile([128, 4], F32, tag="mx4")
                        for ti in range(ntile):
                            w = min(kw - 1024 * ti, 1024)
                            nc.scalar.activation(
                                out=e_t[:, 1024 * ti : 1024 * ti + w],
                                in_=s_tiles[ti][:, :w],
                                func=mybir.ActivationFunctionType.Exp,
                                bias=nmS[:],
                                scale=SCALE,
                                accum_out=rcp[:, ti : ti + 1],
                            )
                        rtot = smp.tile([128, 1], F32, tag="mx")
                        if ntile > 1:
                            nc.vector.reduce_sum(
                                out=rtot[:], in_=rcp[:, :ntile], axis=mybir.AxisListType.X
                            )
                        else:
                            rtot = rcp[:, 0:1]
                        rinv = smp.tile([128, 1], F32, tag="mx")
                        nc.vector.reciprocal(out=rinv[:], in_=rtot[:])
                        d_h = dsm.tile([128, 128], BF16, tag="D")
                        nc.vector.tensor_scalar_mul(d_h[:], ident_bf[:], rinv[:])
                        if pend_diag is not None:
                            pd, pe, ph = pend_diag
                            for kc in range(kwc):
                                nc.tensor.matmul(
                                    P_acc[:, kc * 512 : (kc + 1) * 512],
                                    pd[:],
                                    pe[:, kc * 512 : (kc + 1) * 512],
                                    start=(ph == 0),
                                    stop=False,
                                    skip_group_check=True,
                                )
                        pend_diag = (d_h, e_t, h)
                    pd, pe, ph = pend_diag
                    for kc in range(kwc):
                        nc.tensor.matmul(
                            P_acc[:, kc * 512 : (kc + 1) * 512],
                            pd[:],
                            pe[:, kc * 512 : (kc + 1) * 512],
                            start=False,
                            stop=True,
                            skip_group_check=True,
                        )
                    nc.scalar.copy(out=P_sb[s][:], in_=P_acc[:, :kw])
                    ppp_cm.__exit__(None, None, None)

            # ---------------- transposes: P_sb -> pt ----------------
            with tc.tile_pool(name="tr_ps", bufs=4, space="PSUM") as trp:
                for s in range(4):
                    for kb in range(4 * (s + 1)):
                        tp = trp.tile([128, 128], BF16, tag="tr")
                        nc.tensor.transpose(
                            tp[:], P_sb[s][:, kb * 128 : (kb + 1) * 128], ident_bf[:]
                        )
                        nc.vector.tensor_copy(
                            out=pt[:, kb, s * 128 : (s + 1) * 128], in_=tp[:]
                        )

            # ---------------- Phase D: OT, Y ----------------
            with (
                tc.tile_pool(name="dpool", bufs=1) as dp,
                tc.tile_pool(name="ysb", bufs=4) as ysbp,
                tc.tile_pool(name="vfs", bufs=3) as vfsp,
                tc.tile_pool(name="wos", bufs=2) as wosp,
                tc.tile_pool(name="ot_ps", bufs=4, space="PSUM") as otbp,
                tc.tile_pool(name="y_ps", bufs=4, space="PSUM") as yps,
            ):
                ot = dp.tile([128, NT, 512], BF16)  # [dv-part, dvt, own-q]
                for dvt in range(NT):
                    vf_t = vfsp.tile([128, NT, 128], BF16, tag="vf")
                    vsrc = v_ag[dvt // 4].rearrange("(t p) d -> p t d", p=128)
                    nc.gpsimd.dma_start(
                        vf_t[:],
                        vsrc[:, :, (dvt % 4) * 128 : (dvt % 4) * 128 + 128],
                    )
                    po = otbp.tile([128, 512], F32, tag="OTB")
                    for kt in range(NT):
                        c0 = 128 * (kt // 4)
                        nc.tensor.matmul(
                            po[:, c0:512],
                            vf_t[:, kt, :],
                            pt[:, kt, c0:512],
                            start=(kt == 0),
                            stop=(kt == NT - 1),
                        )
                    nc.vector.tensor_copy(out=ot[:, dvt, :], in_=po[:])
                for nch in range(4):
                    wo_nch = wosp.tile([128, NT, 512], BF16, tag="wos")
                    for dvt in range(NT):
                        nc.sync.dma_start(
                            wo_nch[:, dvt, :],
                            wo[dvt * 128 : (dvt + 1) * 128, nch * 512 : nch * 512 + 512],
                        )
                    for qb in range(4):
                        yp = yps.tile([128, 512], F32, tag="Y")
                        for dvt in range(NT):
                            nc.tensor.matmul(
                                yp[:],
                                ot[:, dvt, qb * 128 : (qb + 1) * 128],
                                wo_nch[:, dvt, :],
                                start=(dvt == 0),
                                stop=(dvt == NT - 1),
                            )
                        y_sb = ysbp.tile([128, 512], F32, tag="ysb")
                        nc.scalar.copy(out=y_sb[:], in_=yp[:])
                        nc.sync.dma_start(
                            out[qb * 128 : (qb + 1) * 128, nch * 512 : nch * 512 + 512],
                            y_sb[:],
                        )

    nc.compile()
    return nc


_NC_CACHE = None


def _make_mask(r):
    rows = np.arange(128)[:, None]
    cols = np.arange(512)[None, :]
    return np.where(cols <= rows + 128 * r, 0.0, NEG).astype(np.float32)


def kernel(x, W_q, W_k, W_v, W_o):
    global _NC_CACHE
    x = np.asarray(x, dtype=np.float32)
    W_q = np.asarray(W_q, dtype=np.float32)
    W_k = np.asarray(W_k, dtype=np.float32)
    W_v = np.asarray(W_v, dtype=np.float32)
    W_o = np.asarray(W_o, dtype=np.float32)
    if _NC_CACHE is None:
        _NC_CACHE = build()
    nc = _NC_CACHE

    wo_bf = W_o.astype(ml_dtypes.bfloat16)
    xT = [np.ascontiguousarray(x[g].T) for g in range(2)]
    xT_bf = [t.astype(ml_dtypes.bfloat16) for t in xT]
    masks = [_make_mask(r) for r in range(4)]
    wqf = np.ascontiguousarray(W_q)
    own_cols = [
        np.concatenate([np.arange((4 * s + r) * 128, (4 * s + r) * 128 + 128) for s in range(4)])
        for r in range(4)
    ]
    in_maps = []
    for c in range(8):
        g, r = divmod(c, 4)
        in_maps.append(
            {
                "x": xT[g],
                "xbf": xT_bf[g],
                "x_own": np.ascontiguousarray(xT[g][:, own_cols[r]]),
                "wqf": wqf,
                "wk": np.ascontiguousarray(W_k[:, 512 * r : 512 * (r + 1)]),
                "wv": np.ascontiguousarray(W_v[:, 512 * r : 512 * (r + 1)]).astype(
                    ml_dtypes.bfloat16
                ),
                "wo": wo_bf,
                "mask": masks[r],
            }
        )
    res = run_bass_kernel_spmd(nc, in_maps, core_ids=list(range(8)))
    Y = np.empty((2, S, D), dtype=np.float32)
    for c in range(8):
        g, r = divmod(c, 4)
        o = res.results[c]["out"]
        for s_idx in range(4):
            t = 4 * s_idx + r
            Y[g, t * 128 : (t + 1) * 128, :] = o[s_idx * 128 : (s_idx + 1) * 128, :]
    return Y


# revision 16
# speedup vs baseline: 1.0696x; 1.0696x over previous
"""Distributed attention kernel for 8 trn2 NeuronCores.

Reference semantics (B=2, S=2048, D=2048, H=16, dh=128):
  q = x@W_q, k = x@W_k  (per-head split), v = x@W_v (full width)
  scores = q@k^T per head; (scores + triu(-1e9)) * 1/sqrt(dh); softmax
  out = (sum_h probs_h) @ v @ W_o        <- heads summed, v full width

Sharding: 2 groups of 4 cores (batch parallel); within a group, rank r
projects heads {4r..4r+3} of k (cols of W_k) and cols [512r, 512r+512)
of W_v.  kT is AllGathered (f32) and v is AllGathered (bf16).  Each
rank computes qT for all 16 heads of its own 4 interleaved q-tiles
{r, 4+r, 8+r, 12+r} (load-balanced under the causal mask) locally
from host-gathered inputs x_own (own 512 q-columns of x^T) and wqf
(full W_q), then computes scores/softmax/P/(P@v)/W_o end-to-end for
its own 512 q rows.  P never leaves SBUF: summed over all 16 heads
in PSUM via diag(1/rowsum) matmuls, PE-transposed, then contracted
with v and W_o.  No ReduceScatter, no P round-trip.

The program is rank-agnostic (required: one NEFF for all 8 cores).
Rank enters only through input data: which W_k/W_v columns are fed,
x_own, and the causal `mask` tensor (diag block at col 128*r of the
last 512-chunk).  kwc(own[s]) = s+1 for every rank, so causal loop
bounds are identical across ranks.

Precision: score path (x@Wq, x@Wk, q@k^T) in float32r (TF32-like,
full PE rate at N>=256); softmax in f32; E/P in bf16; v/P^T/O/W_o in
bf16 with f32 PSUM accumulation.
"""

import math

import numpy as np
import ml_dtypes

import concourse.bass as bass
import concourse.mybir as mybir
import concourse.tile as tile
from concourse import bacc
from concourse.bass_utils import run_bass_kernel_spmd
from concourse.masks import make_identity

F32 = mybir.dt.float32
F32R = mybir.dt.float32r
BF16 = mybir.dt.bfloat16

S = 2048
D = 2048
DH = 128
NT = S // 128  # 16 q/k tiles
SCALE = 1.0 / math.sqrt(DH)
GROUPS = [[0, 1, 2, 3], [4, 5, 6, 7]]
NEG = -1e9


def build():
    nc = bacc.Bacc("TRN2", target_bir_lowering=False, debug=False, num_devices=8)

    x = nc.declare_dram_parameter("x", [D, S], F32R, isOutput=False)  # x TRANSPOSED
    xbf = nc.declare_dram_parameter("xbf", [D, S], BF16, isOutput=False)
    x_own = nc.declare_dram_parameter("x_own", [D, 512], F32R, isOutput=False)
    wqf = nc.declare_dram_parameter("wqf", [D, D], F32R, isOutput=False)
    wk = nc.declare_dram_parameter("wk", [D, 512], F32R, isOutput=False)
    wv = nc.declare_dram_parameter("wv", [D, 512], BF16, isOutput=False)
    wo = nc.declare_dram_parameter("wo", [D, D], BF16, isOutput=False)
    mask_in = nc.declare_dram_parameter("mask", [128, 512], F32R, isOutput=False)
    out = nc.declare_dram_parameter("out", [512, D], F32, isOutput=True)

    # internal DRAM for collectives
    kt_l = nc.dram_tensor("kt_l", [128, 4 * S], F32R)  # [dh, hh*S + k]
    kt_ag = nc.dram_tensor("kt_ag", [4, 128, 4 * S], F32R)
    v_local = nc.dram_tensor("v_local", [S, 512], BF16)
    v_ag = nc.dram_tensor("v_ag", [4, S, 512], BF16)

    with tile.TileContext(nc) as tc:
        with tc.tile_pool(name="const", bufs=1) as cst:
            ident = cst.tile([128, 128], F32)
            make_identity(nc, ident)
            ident_bf = cst.tile([128, 128], BF16)
            nc.vector.tensor_copy(out=ident_bf[:], in_=ident[:])
            mask32 = cst.tile([128, 512], F32R)
            nc.sync.dma_start(mask32[:], mask_in[:])
            mask_bf = cst.tile([128, 512], BF16)
            nc.vector.tensor_copy(out=mask_bf[:], in_=mask32[:])
            # P^T for own q columns, bf16: [k-part, kt, own-q(slab-ordered)]
            pt = cst.tile([128, NT, 512], BF16)
            # P (pre-transpose), per own slab s, exact causal width 512*(s+1)
            P_sb = [cst.tile([128, 512 * (s + 1)], BF16, name=f"Psb{s}") for s in range(4)]

            # qT for own q rows, all 16 heads: [dh-part, h, own-q(slab order)]
            qT_own = cst.tile([128, 16, 512], F32R, name="qT_own")

            # ---------------- Phase A: k projection ----------------
            with (
                tc.tile_pool(name="xt_pool", bufs=1) as xtp,
                tc.tile_pool(name="w_pool", bufs=1) as wpp,
                tc.tile_pool(name="stage", bufs=6) as stp,
                tc.tile_pool(name="ab_ps", bufs=8, space="PSUM") as pjp,
            ):
                wk_sb = wpp.tile([128, NT, 512], F32R, name="wk_sb")
                for Dt in range(NT):
                    nc.sync.dma_start(wk_sb[:, Dt, :], wk[Dt * 128 : (Dt + 1) * 128, :])
                for sh in range(2):  # S half
                    s0 = sh * 1024
                    xt = xtp.tile([128, NT, 1024], F32R, tag="xt", name=f"xt{sh}")
                    psums = [
                        pjp.tile([128, 512], F32, tag="ps512", name=f"proj{_j}")
                        for _j in range(8)
                    ]
                    for Dt in range(NT):
                        nc.sync.dma_start(
                            xt[:, Dt, :],
                            x[Dt * 128 : (Dt + 1) * 128, s0 : s0 + 1024],
                        )
                        for j in range(8):
                            hh, qc = divmod(j, 2)
                            nc.tensor.matmul(
                                psums[j][:],
                                wk_sb[:, Dt, hh * 128 : (hh + 1) * 128],
                                xt[:, Dt, qc * 512 : (qc + 1) * 512],
                                start=(Dt == 0),
                                stop=(Dt == NT - 1),
                            )
                    for j in range(8):
                        hh, qc = divmod(j, 2)
                        st = stp.tile([128, 512], F32R, tag="st")
                        nc.vector.tensor_copy(out=st[:], in_=psums[j][:])
                        nc.sync.dma_start(
                            kt_l[
                                :,
                                hh * S + s0 + qc * 512 : hh * S + s0 + (qc + 1) * 512,
                            ],
                            st[:],
                        )

            nc.gpsimd.collective_compute(
                "AllGather",
                mybir.AluOpType.bypass,
                ins=[kt_l[:]],
                outs=[kt_ag[:]],
                replica_groups=GROUPS,
            )

            # ---------------- Phase Q: local q projection ----------------
            # qT_own[dh, h, s*128+qq] = sum_D wqf[D, h*128+dh] * x_own[D, s*128+qq]
            with (
                tc.tile_pool(name="xo_pool", bufs=1) as xop,
                tc.tile_pool(name="wqf_pool", bufs=6) as wfp,
                tc.tile_pool(name="q_ps", bufs=8, space="PSUM") as qps,
            ):
                xo = xop.tile([128, NT, 512], F32R, name="xo")
                for Dt in range(NT):
                    nc.sync.dma_start(xo[:, Dt, :], x_own[Dt * 128 : (Dt + 1) * 128, :])
                for hp in range(2):  # 8 heads per pass
                    psums = [
                        qps.tile([128, 512], F32, tag="qp", name=f"qproj{_j}")
                        for _j in range(8)
                    ]
                    for Dt in range(NT):
                        wf_t = wfp.tile([128, 1024], F32R, tag="wf")
                        nc.sync.dma_start(
                            wf_t[:],
                            wqf[
                                Dt * 128 : (Dt + 1) * 128,
                                hp * 1024 : (hp + 1) * 1024,
                            ],
                        )
                        for j in range(8):
                            nc.tensor.matmul(
                                psums[j][:],
                                wf_t[:, j * 128 : (j + 1) * 128],
                                xo[:, Dt, :],
                                start=(Dt == 0),
                                stop=(Dt == NT - 1),
                            )
                    for j in range(8):
                        h = hp * 8 + j
                        nc.vector.tensor_copy(out=qT_own[:, h, :], in_=psums[j][:])

            # ---------------- Phase B: v projection ----------------
            with (
                tc.tile_pool(name="wv_pool", bufs=1) as wvp,
                tc.tile_pool(name="bstage", bufs=6) as bstp,
                tc.tile_pool(name="b_ps", bufs=8, space="PSUM") as bjp,
            ):
                wv_sb = wvp.tile([128, NT, 512], BF16, name="wv_sb")
                for Dt in range(NT):
                    nc.sync.dma_start(wv_sb[:, Dt, :], wv[Dt * 128 : (Dt + 1) * 128, :])
                for sh in range(2):
                    s0 = sh * 1024
                    psums = [
                        bjp.tile([128, 512], F32, tag="ps512", name=f"projv{_j}")
                        for _j in range(8)
                    ]
                    for Dt in range(NT):
                        xb_t = bstp.tile([128, 1024], BF16, tag="xb")
                        nc.sync.dma_start(
                            xb_t[:], xbf[Dt * 128 : (Dt + 1) * 128, s0 : s0 + 1024]
                        )
                        for j in range(8):
                            nc.tensor.matmul(
                                psums[j][:],
                                xb_t[:, j * 128 : (j + 1) * 128],
                                wv_sb[:, Dt, :],
                                start=(Dt == 0),
                                stop=(Dt == NT - 1),
                            )
                    for j in range(8):
                        kt = sh * 8 + j
                        v_sb = bstp.tile([128, 512], BF16, tag="vsb")
                        nc.vector.tensor_copy(out=v_sb[:], in_=psums[j][:])
                        nc.sync.dma_start(v_local[kt * 128 : (kt + 1) * 128, :], v_sb[:])

            nc.gpsimd.collective_compute(
                "AllGather",
                mybir.AluOpType.bypass,
                ins=[v_local[:]],
                outs=[v_ag[:]],
                replica_groups=GROUPS,
            )

            # ---------------- Phase C: scores / softmax / P ----------------
            with (
                tc.tile_pool(name="kt_pool", bufs=1) as ktp,
                tc.tile_pool(name="epool", bufs=2) as ep,
                tc.tile_pool(name="small", bufs=32) as smp,
                tc.tile_pool(name="dsm", bufs=4) as dsm,
                tc.tile_pool(name="sc_ps", bufs=2, space="PSUM") as scp,
            ):
                kT = ktp.tile([128, 16, S], F32R, name="kT_sb")
                for kc in range(4):
                    for hg in range(4):
                        for hh in range(4):
                            nc.gpsimd.dma_start(
                                kT[:, 4 * hg + hh, kc * 512 : (kc + 1) * 512],
                                kt_ag[hg][:, hh * S + kc * 512 : hh * S + (kc + 1) * 512],
                            )
                for s in range(4):
                    kwc = s + 1
                    kw = 512 * kwc
                    ntile = (kw + 1023) // 1024
                    ppp_cm = tc.tile_pool(name=f"p_ps{s}", bufs=1, space="PSUM")
                    ppp = ppp_cm.__enter__()
                    P_acc = ppp.tile([128, kw], F32, tag="P", name=f"Pacc{s}")
                    pend_diag = None
                    for h in range(16):
                        e_t = ep.tile([128, 2048], BF16, tag="E")
                        s_tiles = [
                            scp.tile([128, 1024], F32, tag="S", name=f"sch{_j}")
                            for _j in range(ntile)
                        ]
                        dtid = (kw - 512) // 1024
                        doff = (kw - 512) % 1024
                        for kc in range(kwc):
                            last = kc == kwc - 1
                            nc.tensor.matmul(
                                s_tiles[kc // 2][
                                    :, (kc % 2) * 512 : (kc % 2) * 512 + 512
                                ],
                                qT_own[:, h, s * 128 : (s + 1) * 128],
                                kT[:, h, kc * 512 : (kc + 1) * 512],
                                start=True,
                                stop=not last,
                            )
                            if last:
                                # causal mask folded into the PSUM chain
                                nc.tensor.matmul(
                                    s_tiles[dtid][:, doff : doff + 512],
                                    ident_bf[:],
                                    mask_bf[:],
                                    start=False,
                                    stop=True,
                                    skip_group_check=True,
                                )
                        mxp = smp.tile([128, 4], F32, tag="mx4")
                        for ti in range(ntile):
                            w = min(kw - 1024 * ti, 1024)
                            nc.vector.reduce_max(
                                out=mxp[:, ti : ti + 1],
                                in_=s_tiles[ti][:, :w],
                                axis=mybir.AxisListType.X,
                            )
                        nmS = smp.tile([128, 1], F32, tag="mx")
                        if ntile > 1:
                            mx = smp.tile([128, 1], F32, tag="mx")
                            nc.vector.reduce_max(
                                out=mx[:], in_=mxp[:, :ntile], axis=mybir.AxisListType.X
                            )
                        else:
                            mx = mxp[:, 0:1]
                        nc.vector.tensor_scalar_mul(nmS[:], mx[:], -SCALE)
                        rcp = smp.tile([128, 4], F32, tag="mx4")
                        for ti in range(ntile):
                            w = min(kw - 1024 * ti, 1024)
                            nc.scalar.activation(
                                out=e_t[:, 1024 * ti : 1024 * ti + w],
                                in_=s_tiles[ti][:, :w],
                                func=mybir.ActivationFunctionType.Exp,
                                bias=nmS[:],
                                scale=SCALE,
                                accum_out=rcp[:, ti : ti + 1],
                            )
                        rtot = smp.tile([128, 1], F32, tag="mx")
                        if ntile > 1:
                            nc.vector.reduce_sum(
                                out=rtot[:], in_=rcp[:, :ntile], axis=mybir.AxisListType.X
                            )
                        else:
                            rtot = rcp[:, 0:1]
                        rinv = smp.tile([128, 1], F32, tag="mx")
                        nc.vector.reciprocal(out=rinv[:], in_=rtot[:])
                        d_h = dsm.tile([128, 128], BF16, tag="D")
                        nc.vector.tensor_scalar_mul(d_h[:], ident_bf[:], rinv[:])
                        if pend_diag is not None:
                            pd, pe, ph = pend_diag
                            for kc in range(kwc):
                                nc.tensor.matmul(
                                    P_acc[:, kc * 512 : (kc + 1) * 512],
                                    pd[:],
                                    pe[:, kc * 512 : (kc + 1) * 512],
                                    start=(ph == 0),
                                    stop=False,
                                    skip_group_check=True,
                                )
                        pend_diag = (d_h, e_t, h)
                    pd, pe, ph = pend_diag
                    for kc in range(kwc):
                        nc.tensor.matmul(
                            P_acc[:, kc * 512 : (kc + 1) * 512],
                            pd[:],
                            pe[:, kc * 512 : (kc + 1) * 512],
                            start=False,
                            stop=True,
                            skip_group_check=True,
                        )
                    nc.scalar.copy(out=P_sb[s][:], in_=P_acc[:, :kw])
                    ppp_cm.__exit__(None, None, None)

            # ---------------- transposes: P_sb -> pt ----------------
            with tc.tile_pool(name="tr_ps", bufs=4, space="PSUM") as trp:
                for s in range(4):
                    for kb in range(4 * (s + 1)):
                        tp = trp.tile([128, 128], BF16, tag="tr")
                        nc.tensor.transpose(
                            tp[:], P_sb[s][:, kb * 128 : (kb + 1) * 128], ident_bf[:]
                        )
                        nc.vector.tensor_copy(
                            out=pt[:, kb, s * 128 : (s + 1) * 128], in_=tp[:]
                        )

            # ---------------- Phase D: OT, Y ----------------
            with (
                tc.tile_pool(name="dpool", bufs=1) as dp,
                tc.tile_pool(name="ysb", bufs=4) as ysbp,
                tc.tile_pool(name="vfs", bufs=3) as vfsp,
                tc.tile_pool(name="wos", bufs=2) as wosp,
                tc.tile_pool(name="ot_ps", bufs=4, space="PSUM") as otbp,
                tc.tile_pool(name="y_ps", bufs=4, space="PSUM") as yps,
            ):
                ot = dp.tile([128, NT, 512], BF16)  # [dv-part, dvt, own-q]
                for dvt in range(NT):
                    vf_t = vfsp.tile([128, NT, 128], BF16, tag="vf")
                    vsrc = v_ag[dvt // 4].rearrange("(t p) d -> p t d", p=128)
                    nc.gpsimd.dma_start(
                        vf_t[:],
                        vsrc[:, :, (dvt % 4) * 128 : (dvt % 4) * 128 + 128],
                    )
                    po = otbp.tile([128, 512], F32, tag="OTB")
                    for kt in range(NT):
                        c0 = 128 * (kt // 4)
                        nc.tensor.matmul(
                            po[:, c0:512],
                            vf_t[:, kt, :],
                            pt[:, kt, c0:512],
                            start=(kt == 0),
                            stop=(kt == NT - 1),
                        )
                    nc.vector.tensor_copy(out=ot[:, dvt, :], in_=po[:])
                for nch in range(4):
                    wo_nch = wosp.tile([128, NT, 512], BF16, tag="wos")
                    for dvt in range(NT):
                        nc.sync.dma_start(
                            wo_nch[:, dvt, :],
                            wo[dvt * 128 : (dvt + 1) * 128, nch * 512 : nch * 512 + 512],
                        )
                    for qb in range(4):
                        yp = yps.tile([128, 512], F32, tag="Y")
                        for dvt in range(NT):
                            nc.tensor.matmul(
                                yp[:],
                                ot[:, dvt, qb * 128 : (qb + 1) * 128],
                                wo_nch[:, dvt, :],
                                start=(dvt == 0),
                                stop=(dvt == NT - 1),
                            )
                        y_sb = ysbp.tile([128, 512], F32, tag="ysb")
                        nc.scalar.copy(out=y_sb[:], in_=yp[:])
                        nc.sync.dma_start(
                            out[qb * 128 : (qb + 1) * 128, nch * 512 : nch * 512 + 512],
                            y_sb[:],
                        )

    nc.compile()
    return nc


_NC_CACHE = None


def _make_mask(r):
    rows = np.arange(128)[:, None]
    cols = np.arange(512)[None, :]
    return np.where(cols <= rows + 128 * r, 0.0, NEG).astype(np.float32)


def kernel(x, W_q, W_k, W_v, W_o):
    global _NC_CACHE
    x = np.asarray(x, dtype=np.float32)
    W_q = np.asarray(W_q, dtype=np.float32)
    W_k = np.asarray(W_k, dtype=np.float32)
    W_v = np.asarray(W_v, dtype=np.float32)
    W_o = np.asarray(W_o, dtype=np.float32)
    if _NC_CACHE is None:
        _NC_CACHE = build()
    nc = _NC_CACHE

    wo_bf = W_o.astype(ml_dtypes.bfloat16)
    xT = [np.ascontiguousarray(x[g].T) for g in range(2)]
    xT_bf = [t.astype(ml_dtypes.bfloat16) for t in xT]
    masks = [_make_mask(r) for r in range(4)]
    wqf = np.ascontiguousarray(W_q)
    own_cols = [
        np.concatenate([np.arange((4 * s + r) * 128, (4 * s + r) * 128 + 128) for s in range(4)])
        for r in range(4)
    ]
    in_maps = []
    for c in range(8):
        g, r = divmod(c, 4)
        in_maps.append(
            {
                "x": xT[g],
                "xbf": xT_bf[g],
                "x_own": np.ascontiguousarray(xT[g][:, own_cols[r]]),
                "wqf": wqf,
                "wk": np.ascontiguousarray(W_k[:, 512 * r : 512 * (r + 1)]),
                "wv": np.ascontiguousarray(W_v[:, 512 * r : 512 * (r + 1)]).astype(
                    ml_dtypes.bfloat16
                ),
                "wo": wo_bf,
                "mask": masks[r],
            }
        )
    res = run_bass_kernel_spmd(nc, in_maps, core_ids=list(range(8)))
    Y = np.empty((2, S, D), dtype=np.float32)
    for c in range(8):
        g, r = divmod(c, 4)
        o = res.results[c]["out"]
        for s_idx in range(4):
            t = 4 * s_idx + r
            Y[g, t * 128 : (t + 1) * 128, :] = o[s_idx * 128 : (s_idx + 1) * 128, :]
    return Y


# revision 17
# speedup vs baseline: 1.1506x; 1.0757x over previous
"""Distributed attention kernel for 8 trn2 NeuronCores.

Reference semantics (B=2, S=2048, D=2048, H=16, dh=128):
  q = x@W_q, k = x@W_k  (per-head split), v = x@W_v (full width)
  scores = q@k^T per head; (scores + triu(-1e9)) * 1/sqrt(dh); softmax
  out = (sum_h probs_h) @ v @ W_o        <- heads summed, v full width

Sharding: 2 groups of 4 cores (batch parallel); within a group, rank r
projects heads {4r..4r+3} of k (cols of W_k) and cols [512r, 512r+512)
of W_v.  kT is AllGathered (f32) and v is AllGathered (bf16).  Each
rank computes qT for all 16 heads of its own 4 interleaved q-tiles
{r, 4+r, 8+r, 12+r} (load-balanced under the causal mask) locally
from host-gathered inputs x_own (own 512 q-columns of x^T) and wqf
(full W_q), then computes scores/softmax/P/(P@v)/W_o end-to-end for
its own 512 q rows.  P never leaves SBUF: summed over all 16 heads
in PSUM via diag(1/rowsum) matmuls, PE-transposed, then contracted
with v and W_o.  No ReduceScatter, no P round-trip.

The program is rank-agnostic (required: one NEFF for all 8 cores).
Rank enters only through input data: which W_k/W_v columns are fed,
x_own, and the causal `mask` tensor (diag block at col 128*r of the
last 512-chunk).  kwc(own[s]) = s+1 for every rank, so causal loop
bounds are identical across ranks.

Precision: score path (x@Wq, x@Wk, q@k^T) in float32r (TF32-like,
full PE rate at N>=256); softmax in f32; E/P in bf16; v/P^T/O/W_o in
bf16 with f32 PSUM accumulation.
"""

import math

import numpy as np
import ml_dtypes

import concourse.bass as bass
import concourse.mybir as mybir
import concourse.tile as tile
from concourse import bacc
from concourse.bass_utils import run_bass_kernel_spmd
from concourse.masks import make_identity

F32 = mybir.dt.float32
F32R = mybir.dt.float32r
BF16 = mybir.dt.bfloat16

S = 2048
D = 2048
DH = 128
NT = S // 128  # 16 q/k tiles
SCALE = 1.0 / math.sqrt(DH)
GROUPS = [[0, 1, 2, 3], [4, 5, 6, 7]]
NEG = -1e9


def build():
    nc = bacc.Bacc("TRN2", target_bir_lowering=False, debug=False, num_devices=8)

    x = nc.declare_dram_parameter("x", [D, S], F32R, isOutput=False)  # x TRANSPOSED
    xbf = nc.declare_dram_parameter("xbf", [D, S], BF16, isOutput=False)
    x_own = nc.declare_dram_parameter("x_own", [D, 512], F32R, isOutput=False)
    wqf = nc.declare_dram_parameter("wqf", [D, D], F32R, isOutput=False)
    wk = nc.declare_dram_parameter("wk", [D, 512], F32R, isOutput=False)
    wv = nc.declare_dram_parameter("wv", [D, 512], BF16, isOutput=False)
    wo = nc.declare_dram_parameter("wo", [D, D], BF16, isOutput=False)
    mask_in = nc.declare_dram_parameter("mask", [128, 512], F32R, isOutput=False)
    out = nc.declare_dram_parameter("out", [512, D], F32, isOutput=True)

    # internal DRAM for collectives
    kt_l = nc.dram_tensor("kt_l", [128, 4 * S], F32R)  # [dh, hh*S + k]
    kt_ag = nc.dram_tensor("kt_ag", [4, 128, 4 * S], F32R)
    v_local = nc.dram_tensor("v_local", [S, 512], BF16)
    v_ag = nc.dram_tensor("v_ag", [4, S, 512], BF16)

    with tile.TileContext(nc) as tc:
        with tc.tile_pool(name="const", bufs=1) as cst:
            ident = cst.tile([128, 128], F32)
            make_identity(nc, ident)
            ident_bf = cst.tile([128, 128], BF16)
            nc.vector.tensor_copy(out=ident_bf[:], in_=ident[:])
            mask32 = cst.tile([128, 512], F32R)
            nc.sync.dma_start(mask32[:], mask_in[:])
            mask_bf = cst.tile([128, 512], BF16)
            nc.vector.tensor_copy(out=mask_bf[:], in_=mask32[:])
            # P^T for own q columns, bf16: [k-part, kt, own-q(slab-ordered)]
            pt = cst.tile([128, NT, 512], BF16)
            # P (pre-transpose), per own slab s, exact causal width 512*(s+1)
            P_sb = [cst.tile([128, 512 * (s + 1)], BF16, name=f"Psb{s}") for s in range(4)]

            # qT for own q rows, all 16 heads: [dh-part, h, own-q(slab order)]
            qT_own = cst.tile([128, 16, 512], F32R, name="qT_own")

            # ---------------- Phase A: k projection ----------------
            with (
                tc.tile_pool(name="xt_pool", bufs=1) as xtp,
                tc.tile_pool(name="w_pool", bufs=1) as wpp,
                tc.tile_pool(name="stage", bufs=6) as stp,
                tc.tile_pool(name="ab_ps", bufs=8, space="PSUM") as pjp,
            ):
                wk_sb = wpp.tile([128, NT, 512], F32R, name="wk_sb")
                for Dt in range(NT):
                    nc.sync.dma_start(wk_sb[:, Dt, :], wk[Dt * 128 : (Dt + 1) * 128, :])
                for sh in range(2):  # S half
                    s0 = sh * 1024
                    xt = xtp.tile([128, NT, 1024], F32R, tag="xt", name=f"xt{sh}")
                    psums = [
                        pjp.tile([128, 512], F32, tag="ps512", name=f"proj{_j}")
                        for _j in range(8)
                    ]
                    for Dt in range(NT):
                        nc.sync.dma_start(
                            xt[:, Dt, :],
                            x[Dt * 128 : (Dt + 1) * 128, s0 : s0 + 1024],
                        )
                        for j in range(8):
                            hh, qc = divmod(j, 2)
                            nc.tensor.matmul(
                                psums[j][:],
                                wk_sb[:, Dt, hh * 128 : (hh + 1) * 128],
                                xt[:, Dt, qc * 512 : (qc + 1) * 512],
                                start=(Dt == 0),
                                stop=(Dt == NT - 1),
                            )
                    for j in range(8):
                        hh, qc = divmod(j, 2)
                        st = stp.tile([128, 512], F32R, tag="st")
                        nc.vector.tensor_copy(out=st[:], in_=psums[j][:])
                        nc.sync.dma_start(
                            kt_l[
                                :,
                                hh * S + s0 + qc * 512 : hh * S + s0 + (qc + 1) * 512,
                            ],
                            st[:],
                        )

            nc.gpsimd.collective_compute(
                "AllGather",
                mybir.AluOpType.bypass,
                ins=[kt_l[:]],
                outs=[kt_ag[:]],
                replica_groups=GROUPS,
            )

            # ---------------- Phase Q: local q projection ----------------
            # qT_own[dh, h, s*128+qq] = sum_D wqf[D, h*128+dh] * x_own[D, s*128+qq]
            with (
                tc.tile_pool(name="xo_pool", bufs=1) as xop,
                tc.tile_pool(name="wqf_pool", bufs=6) as wfp,
                tc.tile_pool(name="q_ps", bufs=8, space="PSUM") as qps,
            ):
                xo = xop.tile([128, NT, 512], F32R, name="xo")
                for Dt in range(NT):
                    nc.sync.dma_start(xo[:, Dt, :], x_own[Dt * 128 : (Dt + 1) * 128, :])
                for hp in range(2):  # 8 heads per pass
                    psums = [
                        qps.tile([128, 512], F32, tag="qp", name=f"qproj{_j}")
                        for _j in range(8)
                    ]
                    for Dt in range(NT):
                        wf_t = wfp.tile([128, 1024], F32R, tag="wf")
                        nc.sync.dma_start(
                            wf_t[:],
                            wqf[
                                Dt * 128 : (Dt + 1) * 128,
                                hp * 1024 : (hp + 1) * 1024,
                            ],
                        )
                        for j in range(8):
                            nc.tensor.matmul(
                                psums[j][:],
                                wf_t[:, j * 128 : (j + 1) * 128],
                                xo[:, Dt, :],
                                start=(Dt == 0),
                                stop=(Dt == NT - 1),
                            )
                    for j in range(8):
                        h = hp * 8 + j
                        nc.vector.tensor_copy(out=qT_own[:, h, :], in_=psums[j][:])

            # ---------------- Phase B: v projection ----------------
            with (
                tc.tile_pool(name="wv_pool", bufs=1) as wvp,
                tc.tile_pool(name="bstage", bufs=6) as bstp,
                tc.tile_pool(name="b_ps", bufs=8, space="PSUM") as bjp,
            ):
                wv_sb = wvp.tile([128, NT, 512], BF16, name="wv_sb")
                for Dt in range(NT):
                    nc.sync.dma_start(wv_sb[:, Dt, :], wv[Dt * 128 : (Dt + 1) * 128, :])
                for sh in range(2):
                    s0 = sh * 1024
                    psums = [
                        bjp.tile([128, 512], F32, tag="ps512", name=f"projv{_j}")
                        for _j in range(8)
                    ]
                    for Dt in range(NT):
                        xb_t = bstp.tile([128, 1024], BF16, tag="xb")
                        nc.sync.dma_start(
                            xb_t[:], xbf[Dt * 128 : (Dt + 1) * 128, s0 : s0 + 1024]
                        )
                        for j in range(8):
                            nc.tensor.matmul(
                                psums[j][:],
                                xb_t[:, j * 128 : (j + 1) * 128],
                                wv_sb[:, Dt, :],
                                start=(Dt == 0),
                                stop=(Dt == NT - 1),
                            )
                    for j in range(8):
                        kt = sh * 8 + j
                        v_sb = bstp.tile([128, 512], BF16, tag="vsb")
                        nc.vector.tensor_copy(out=v_sb[:], in_=psums[j][:])
                        nc.sync.dma_start(v_local[kt * 128 : (kt + 1) * 128, :], v_sb[:])

            # ---------------- Phase C: scores / softmax / P ----------------
            with (
                tc.tile_pool(name="kt_pool", bufs=1) as ktp,
                tc.tile_pool(name="epool", bufs=2) as ep,
                tc.tile_pool(name="small", bufs=32) as smp,
                tc.tile_pool(name="dsm", bufs=4) as dsm,
                tc.tile_pool(name="sc_ps", bufs=2, space="PSUM") as scp,
            ):
                kT = ktp.tile([128, 16, S], F32R, name="kT_sb")
                for kc in range(4):
                    for hg in range(4):
                        for hh in range(4):
                            nc.gpsimd.dma_start(
                                kT[:, 4 * hg + hh, kc * 512 : (kc + 1) * 512],
                                kt_ag[hg][:, hh * S + kc * 512 : hh * S + (kc + 1) * 512],
                            )
                nc.gpsimd.collective_compute(
                    "AllGather",
                    mybir.AluOpType.bypass,
                    ins=[v_local[:]],
                    outs=[v_ag[:]],
                    replica_groups=GROUPS,
                )
                for s in range(4):
                    kwc = s + 1
                    kw = 512 * kwc
                    ntile = (kw + 1023) // 1024
                    ppp_cm = tc.tile_pool(name=f"p_ps{s}", bufs=1, space="PSUM")
                    ppp = ppp_cm.__enter__()
                    P_acc = ppp.tile([128, kw], F32, tag="P", name=f"Pacc{s}")
                    pend_diag = None
                    for h in range(16):
                        e_t = ep.tile([128, 2048], BF16, tag="E")
                        s_tiles = [
                            scp.tile([128, 1024], F32, tag="S", name=f"sch{_j}")
                            for _j in range(ntile)
                        ]
                        dtid = (kw - 512) // 1024
                        doff = (kw - 512) % 1024
                        for kc in range(kwc):
                            last = kc == kwc - 1
                            nc.tensor.matmul(
                                s_tiles[kc // 2][
                                    :, (kc % 2) * 512 : (kc % 2) * 512 + 512
                                ],
                                qT_own[:, h, s * 128 : (s + 1) * 128],
                                kT[:, h, kc * 512 : (kc + 1) * 512],
                                start=True,
                                stop=not last,
                            )
                            if last:
                                # causal mask folded into the PSUM chain
                                nc.tensor.matmul(
                                    s_tiles[dtid][:, doff : doff + 512],
                                    ident_bf[:],
                                    mask_bf[:],
                                    start=False,
                                    stop=True,
                                    skip_group_check=True,
                                )
                        mxp = smp.tile([128, 4], F32, tag="mx4")
                        for ti in range(ntile):
                            w = min(kw - 1024 * ti, 1024)
                            nc.vector.reduce_max(
                                out=mxp[:, ti : ti + 1],
                                in_=s_tiles[ti][:, :w],
                                axis=mybir.AxisListType.X,
                            )
                        nmS = smp.tile([128, 1], F32, tag="mx")
                        if ntile > 1:
                            mx = smp.tile([128, 1], F32, tag="mx")
                            nc.vector.reduce_max(
                                out=mx[:], in_=mxp[:, :ntile], axis=mybir.AxisListType.X
                            )
                        else:
                            mx = mxp[:, 0:1]
                        nc.vector.tensor_scalar_mul(nmS[:], mx[:], -SCALE)
                        rcp = smp.tile([128, 4], F32, tag="mx4")
                        for ti in range(ntile):
                            w = min(kw - 1024 * ti, 1024)
                            nc.scalar.activation(
                                out=e_t[:, 1024 * ti : 1024 * ti + w],
                                in_=s_tiles[ti][:, :w],
                                func=mybir.ActivationFunctionType.Exp,
                                bias=nmS[:],
                                scale=SCALE,
                                accum_out=rcp[:, ti : ti + 1],
                            )
                        rtot = smp.tile([128, 1], F32, tag="mx")
                        if ntile > 1:
                            nc.vector.reduce_sum(
                                out=rtot[:], in_=rcp[:, :ntile], axis=mybir.AxisListType.X
                            )
                        else:
                            rtot = rcp[:, 0:1]
                        rinv = smp.tile([128, 1], F32, tag="mx")
                        nc.vector.reciprocal(out=rinv[:], in_=rtot[:])
                        d_h = dsm.tile([128, 128], BF16, tag="D")
                        nc.vector.tensor_scalar_mul(d_h[:], ident_bf[:], rinv[:])
                        if pend_diag is not None:
                            pd, pe, ph = pend_diag
                            for kc in range(kwc):
                                nc.tensor.matmul(
                                    P_acc[:, kc * 512 : (kc + 1) * 512],
                                    pd[:],
                                    pe[:, kc * 512 : (kc + 1) * 512],
                                    start=(ph == 0),
                                    stop=False,
                                    skip_group_check=True,
                                )
                        pend_diag = (d_h, e_t, h)
                    pd, pe, ph = pend_diag
                    for kc in range(kwc):
                        nc.tensor.matmul(
                            P_acc[:, kc * 512 : (kc + 1) * 512],
                            pd[:],
                            pe[:, kc * 512 : (kc + 1) * 512],
                            start=False,
                            stop=True,
                            skip_group_check=True,
                        )
                    nc.scalar.copy(out=P_sb[s][:], in_=P_acc[:, :kw])
                    ppp_cm.__exit__(None, None, None)

            # ---------------- transposes: P_sb -> pt ----------------
            with tc.tile_pool(name="tr_ps", bufs=4, space="PSUM") as trp:
                for s in range(4):
                    for kb in range(4 * (s + 1)):
                        tp = trp.tile([128, 128], BF16, tag="tr")
                        nc.tensor.transpose(
                            tp[:], P_sb[s][:, kb * 128 : (kb + 1) * 128], ident_bf[:]
                        )
                        nc.vector.tensor_copy(
                            out=pt[:, kb, s * 128 : (s + 1) * 128], in_=tp[:]
                        )

            # ---------------- Phase D: OT, Y ----------------
            with (
                tc.tile_pool(name="dpool", bufs=1) as dp,
                tc.tile_pool(name="ysb", bufs=4) as ysbp,
                tc.tile_pool(name="vfs", bufs=3) as vfsp,
                tc.tile_pool(name="wos", bufs=2) as wosp,
                tc.tile_pool(name="ot_ps", bufs=4, space="PSUM") as otbp,
                tc.tile_pool(name="y_ps", bufs=4, space="PSUM") as yps,
            ):
                ot = dp.tile([128, NT, 512], BF16)  # [dv-part, dvt, own-q]
                for dvt in range(NT):
                    vf_t = vfsp.tile([128, NT, 128], BF16, tag="vf")
                    vsrc = v_ag[dvt // 4].rearrange("(t p) d -> p t d", p=128)
                    nc.gpsimd.dma_start(
                        vf_t[:],
                        vsrc[:, :, (dvt % 4) * 128 : (dvt % 4) * 128 + 128],
                    )
                    po = otbp.tile([128, 512], F32, tag="OTB")
                    for kt in range(NT):
                        c0 = 128 * (kt // 4)
                        nc.tensor.matmul(
                            po[:, c0:512],
                            vf_t[:, kt, :],
                            pt[:, kt, c0:512],
                            start=(kt == 0),
                            stop=(kt == NT - 1),
                        )
                    nc.vector.tensor_copy(out=ot[:, dvt, :], in_=po[:])
                for nch in range(4):
                    wo_nch = wosp.tile([128, NT, 512], BF16, tag="wos")
                    for dvt in range(NT):
                        nc.sync.dma_start(
                            wo_nch[:, dvt, :],
                            wo[dvt * 128 : (dvt + 1) * 128, nch * 512 : nch * 512 + 512],
                        )
                    for qb in range(4):
                        yp = yps.tile([128, 512], F32, tag="Y")
                        for dvt in range(NT):
                            nc.tensor.matmul(
                                yp[:],
                                ot[:, dvt, qb * 128 : (qb + 1) * 128],
                                wo_nch[:, dvt, :],
                                start=(dvt == 0),
                                stop=(dvt == NT - 1),
                            )
                        y_sb = ysbp.tile([128, 512], F32, tag="ysb")
                        nc.scalar.copy(out=y_sb[:], in_=yp[:])
                        nc.sync.dma_start(
                            out[qb * 128 : (qb + 1) * 128, nch * 512 : nch * 512 + 512],
                            y_sb[:],
                        )

    nc.compile()
    return nc


_NC_CACHE = None


def _make_mask(r):
    rows = np.arange(128)[:, None]
    cols = np.arange(512)[None, :]
    return np.where(cols <= rows + 128 * r, 0.0, NEG).astype(np.float32)


def kernel(x, W_q, W_k, W_v, W_o):
    global _NC_CACHE
    x = np.asarray(x, dtype=np.float32)
    W_q = np.asarray(W_q, dtype=np.float32)
    W_k = np.asarray(W_k, dtype=np.float32)
    W_v = np.asarray(W_v, dtype=np.float32)
    W_o = np.asarray(W_o, dtype=np.float32)
    if _NC_CACHE is None:
        _NC_CACHE = build()
    nc = _NC_CACHE

    wo_bf = W_o.astype(ml_dtypes.bfloat16)
    xT = [np.ascontiguousarray(x[g].T) for g in range(2)]
    xT_bf = [t.astype(ml_dtypes.bfloat16) for t in xT]
    masks = [_make_mask(r) for r in range(4)]
    wqf = np.ascontiguousarray(W_q)
    own_cols = [
        np.concatenate([np.arange((4 * s + r) * 128, (4 * s + r) * 128 + 128) for s in range(4)])
        for r in range(4)
    ]
    in_maps = []
    for c in range(8):
        g, r = divmod(c, 4)
        in_maps.append(
            {
                "x": xT[g],
                "xbf": xT_bf[g],
                "x_own": np.ascontiguousarray(xT[g][:, own_cols[r]]),
                "wqf": wqf,
                "wk": np.ascontiguousarray(W_k[:, 512 * r : 512 * (r + 1)]),
                "wv": np.ascontiguousarray(W_v[:, 512 * r : 512 * (r + 1)]).astype(
                    ml_dtypes.bfloat16
                ),
                "wo": wo_bf,
                "mask": masks[r],
            }
        )
    res = run_bass_kernel_spmd(nc, in_maps, core_ids=list(range(8)))
    Y = np.empty((2, S, D), dtype=np.float32)
    for c in range(8):
        g, r = divmod(c, 4)
        o = res.results[c]["out"]
        for s_idx in range(4):
            t = 4 * s_idx + r
            Y[g, t * 128 : (t + 1) * 128, :] = o[s_idx * 128 : (s_idx + 1) * 128, :]
    return Y
